# revision 6
# baseline (speedup 1.0000x reference)
"""Trainium2 Bass kernel v3 for nn_DeltaSynapse.

Reference (D=16 delays, B=8 batch, E=2048 pre, O=2048 post):
    Weff = signs * W                                  (e, o)
    I[b,o] = sum_{d,e} Weff[e,o] * Xd[d,b,e] * delaymap[d,e,o] * (Wshort[d,b,e]+1)

Sharding: O split across 8 cores; replicated (transposed) Xd / Wshort.

v3 vs the 43-us baseline:
1. fp8 DoubleRow matmuls (two e-chunk planes per instruction, PE sums both):
   128 matmuls of ~53 ns instead of 256 bf16 matmuls of ~107 ns.
2. delaymap mask-multiply as bitwise AND on uint32 views: host ships dm as
   0x00/0xFF bytes, m_fp8 = weff8 & mask.  Exact, no fp8-arithmetic slowdown.
3. weff8 = w8 | sign-bit mask (one OR); W==0 exactly where signs==0.
4. DoubleRow M=16 lhsT = [Xd | Xd*Wshort] columns: psum rows 0-7 + 8-15 give
   I without fp8-rounding (1+Wshort); the Xd*Wshort half is masked in place
   (ws & xdmask) after one contiguous DMA of the padded lhsT image.
5. Per-delay pipeline granularity with DMAs round-robined over the SP and ACT
   HWDGE queues; the reps timing loop is 2-unrolled to halve the For_i
   all-engine barrier + pipeline-drain cost.
6. Tail: psum -> SBUF copy, then DRAM out = lo half + accumulating gpsimd DMA
   of the hi half.

Per-core traffic ~9.1 MiB (vs 18.25); measured leg times: DMA ~17 us,
DVE ANDs ~17 us, PE ~7 us, overlapped.
"""

import numpy as np

import concourse.bacc as bacc
import concourse.mybir as mybir
import concourse.tile as tile
from concourse.bass_utils import run_bass_kernel_spmd

D, B, E, O = 16, 8, 2048, 2048
NCORES = 8
OS = O // NCORES  # 256
ET = E // 128  # 16 e-chunks
DB = D * B  # 128

LAST_EXEC_TIME_NS = None
_CACHED_NC = {}

f32 = mybir.dt.float32
f8 = mybir.dt.float8e4
u32 = mybir.dt.uint32


def build_module(reps=1):
    if reps in _CACHED_NC:
        return _CACHED_NC[reps]
    import contextlib

    nc = bacc.Bacc("TRN2", target_bir_lowering=False, debug=False)

    FREE = ET * OS  # 4096 bytes per partition per delay
    dmm = nc.dram_tensor("dmm", (D, 128, ET, OS // 4), u32, kind="ExternalInput").ap()
    w8 = nc.dram_tensor("w8", (128, FREE // 4), u32, kind="ExternalInput").ap()
    sgn = nc.dram_tensor("sgn", (128, FREE // 4), u32, kind="ExternalInput").ap()
    apad = nc.dram_tensor("apad", (128, ET // 2, 2, D, 16), f8, kind="ExternalInput").ap()
    xdm = nc.dram_tensor("xdm", (128, ET * DB // 4), u32, kind="ExternalInput").ap()
    out = nc.dram_tensor("out", (B, OS), f32, kind="ExternalOutput").ap()

    with tile.TileContext(nc) as tc:
        with (
            tc.tile_pool(name="const", bufs=2) as const,
            tc.tile_pool(name="dm", bufs=6) as dmp,
            tc.tile_pool(name="m", bufs=6) as mp,
            tc.tile_pool(name="ps", bufs=2, space="PSUM") as pp,
            tc.tile_pool(name="o", bufs=2) as op,
        ):

            def body():
                w_sb = const.tile([128, FREE // 4], u32, tag="w")
                s_sb = const.tile([128, FREE // 4], u32, tag="s")
                weff = const.tile([128, ET, OS], f8, tag="weff")
                xdm_sb = const.tile([128, ET * DB // 8, 2], u32, tag="xdm")
                a8 = const.tile([128, ET // 2, 2, D, 16], f8, tag="a8")

                nc.scalar.dma_start(out=w_sb[:], in_=w8[:])
                nc.scalar.dma_start(out=s_sb[:], in_=sgn[:])
                nc.scalar.dma_start(out=a8[:], in_=apad[:])
                nc.scalar.dma_start(
                    out=xdm_sb[:], in_=xdm[:].rearrange("p (r two) -> p r two", two=2)
                )

                # weff8 = w8 | sgnmask
                nc.vector.tensor_tensor(
                    weff[:].bitcast(u32), w_sb[:], s_sb[:], mybir.AluOpType.bitwise_or
                )
                # mask lhsT cols 8:16 in place: Xd*Wshort = Wshort & Xd-mask
                a8flat = a8[:].rearrange("p tp j d c -> p (tp j d) c")
                nc.vector.tensor_tensor(
                    a8flat[:, :, B:16].bitcast(u32),
                    a8flat[:, :, B:16].bitcast(u32),
                    xdm_sb[:],
                    mybir.AluOpType.bitwise_and,
                )

                psum = pp.tile([16, OS], f32, tag="ps")
                weff32 = weff[:].bitcast(u32)
                # d=14,15 go to the Pool engine as fp8-value multiplies (their
                # dm bytes are fp8 1.0/0.0); DMA them first so Pool can crunch
                # in the background while DVE handles d=0..13 as mask ANDs.
                POOLD = (14, 15)
                mtiles = {}
                for i, d in enumerate(POOLD):
                    dm = dmp.tile([128, ET, OS // 4], u32, tag="dm")
                    (nc.sync if i % 2 == 0 else nc.scalar).dma_start(
                        out=dm[:], in_=dmm[d]
                    )
                    m = mp.tile([128, ET, OS], f8, tag="m")
                    nc.gpsimd.tensor_mul(m[:], dm[:].bitcast(f8), weff[:])
                    mtiles[d] = m
                for d in range(D):
                    if d not in POOLD:
                        dm = dmp.tile([128, ET, OS // 4], u32, tag="dm")
                        (nc.sync if d % 2 == 0 else nc.scalar).dma_start(
                            out=dm[:], in_=dmm[d]
                        )
                        m = mp.tile([128, ET, OS], f8, tag="m")
                        nc.vector.tensor_tensor(
                            m[:].bitcast(u32), dm[:], weff32,
                            mybir.AluOpType.bitwise_and,
                        )
                        mtiles[d] = m
                    m = mtiles[d]
                    for tp in range(ET // 2):
                        nc.tensor.matmul(
                            psum[:],
                            a8[:, tp, :, d, :],
                            m[:, 2 * tp : 2 * tp + 2, :],
                            start=(d == 0 and tp == 0),
                            stop=(d == D - 1 and tp == ET // 2 - 1),
                            perf_mode=mybir.MatmulPerfMode.DoubleRow,
                        )

                sb16 = op.tile([2 * B, OS], f32, tag="sb16")
                nc.vector.tensor_copy(sb16[:], psum[:])
                hi_sb = op.tile([B, OS], f32, tag="hi")
                nc.gpsimd.dma_start(out=hi_sb[:], in_=sb16[B : 2 * B, :])
                out_sb = op.tile([B, OS], f32, tag="os")
                nc.vector.tensor_add(out_sb[:], sb16[0:B, :], hi_sb[:])
                nc.gpsimd.dma_start(out=out[:], in_=out_sb[:])

            if reps == 1:
                body()
            else:
                UNROLL = 4
                loops, rem = divmod(reps, UNROLL)
                if loops:
                    with tc.For_i(0, loops, 1, hint_engines=(mybir.EngineType.PE,)):
                        for _ in range(UNROLL):
                            body()
                for _ in range(rem):
                    body()

    nc.compile()
    _CACHED_NC[reps] = nc
    return nc


def make_in_maps(W, signs, Xd, Wshort, delaymap):
    """Host-side sharding + transport encoding (value-blind re-encodings):
    fp8/byte-mask casts and layout swizzles only."""
    import ml_dtypes

    f8n = ml_dtypes.float8_e4m3

    def swz(a2d, dtype):  # (E, X) -> [128, ET, X] with e = t*128 + p
        X = a2d.shape[1]
        return np.ascontiguousarray(
            a2d.reshape(ET, 128, X).transpose(1, 0, 2).astype(dtype)
        )

    xdT = np.transpose(Xd, (2, 0, 1)).reshape(E, DB)  # [e, d*B+b]
    wsT = np.transpose(Wshort, (2, 0, 1)).reshape(E, DB)

    def lhst_pack(a2d, dtype):
        # (E, DB) -> [128, ET/2, 2, D, B]: [p][tp][j][d][b] = a[(2tp+j)*128+p, d*B+b]
        a = a2d.reshape(ET // 2, 2, 128, D, B).transpose(2, 0, 1, 3, 4)
        return np.ascontiguousarray(a.astype(dtype))

    apad = np.concatenate(
        [lhst_pack(xdT, f8n), lhst_pack(wsT, f8n)], axis=4
    )  # [128, ET/2, 2, D, 16]: Xd | Wshort
    apad = np.ascontiguousarray(apad)
    xdm = (lhst_pack(xdT, np.uint8) * 255).reshape(128, -1).view(np.uint32)
    xdm = np.ascontiguousarray(xdm)

    in_maps = []
    for c in range(NCORES):
        sl = slice(c * OS, (c + 1) * OS)
        w8 = swz(W[:, sl], f8n).view(np.uint8).reshape(128, -1).view(np.uint32)
        sg = swz((signs[:, sl] < 0).astype(np.float32), np.uint8) * 0x80
        sg = sg.reshape(128, -1).view(np.uint32)
        dmc = np.empty((D, 128, ET, OS // 4), np.uint32)
        for d in range(D):
            scale = 0x38 if d in (14, 15) else 255  # fp8 1.0 for Pool-mult delays
            mb = swz(delaymap[d][:, sl], np.uint8) * scale
            dmc[d] = mb.reshape(128, ET, OS).view(np.uint32).reshape(128, ET, OS // 4)
        in_maps.append(
            {
                "dmm": dmc,
                "w8": np.ascontiguousarray(w8),
                "sgn": np.ascontiguousarray(sg),
                "apad": apad,
                "xdm": xdm,
            }
        )
    return in_maps


def kernel(W, signs, Xd, Wshort, delaymap, trace=False):
    global LAST_EXEC_TIME_NS
    W = np.asarray(W, dtype=np.float32)
    signs = np.asarray(signs, dtype=np.float32)
    Xd = np.asarray(Xd, dtype=np.float32)
    Wshort = np.asarray(Wshort, dtype=np.float32)
    delaymap = np.asarray(delaymap, dtype=np.float32)

    nc = build_module()
    in_maps = make_in_maps(W, signs, Xd, Wshort, delaymap)
    res = run_bass_kernel_spmd(nc, in_maps, core_ids=list(range(NCORES)), trace=trace)
    LAST_EXEC_TIME_NS = res.exec_time_ns
    return np.concatenate([r["out"] for r in res.results], axis=1)


# revision 7
# speedup vs baseline: 1.8247x; 1.8247x over previous
"""Trainium2 Bass kernel v3 for nn_DeltaSynapse.

Reference (D=16 delays, B=8 batch, E=2048 pre, O=2048 post):
    Weff = signs * W                                  (e, o)
    I[b,o] = sum_{d,e} Weff[e,o] * Xd[d,b,e] * delaymap[d,e,o] * (Wshort[d,b,e]+1)

Sharding: O split across 8 cores; replicated (transposed) Xd / Wshort.

v3 vs the 43-us baseline:
1. fp8 DoubleRow matmuls (two e-chunk planes per instruction, PE sums both):
   128 matmuls of ~53 ns instead of 256 bf16 matmuls of ~107 ns.
2. delaymap mask-multiply as bitwise AND on uint32 views: host ships dm as
   0x00/0xFF bytes, m_fp8 = weff8 & mask.  Exact, no fp8-arithmetic slowdown.
3. weff8 = w8 | sign-bit mask (one OR); W==0 exactly where signs==0.
4. DoubleRow M=16 lhsT = [Xd | Xd*Wshort] columns: psum rows 0-7 + 8-15 give
   I without fp8-rounding (1+Wshort); the Xd*Wshort half is masked in place
   (ws & xdmask) after one contiguous DMA of the padded lhsT image.
5. Per-delay pipeline granularity with DMAs round-robined over the SP and ACT
   HWDGE queues; the reps timing loop is 2-unrolled to halve the For_i
   all-engine barrier + pipeline-drain cost.
6. Tail: psum -> SBUF copy, then DRAM out = lo half + accumulating gpsimd DMA
   of the hi half.

Per-core traffic ~9.1 MiB (vs 18.25); measured leg times: DMA ~17 us,
DVE ANDs ~17 us, PE ~7 us, overlapped.
"""

import numpy as np

import concourse.bacc as bacc
import concourse.mybir as mybir
import concourse.tile as tile
from concourse.bass_utils import run_bass_kernel_spmd

D, B, E, O = 16, 8, 2048, 2048
NCORES = 8
OS = O // NCORES  # 256
ET = E // 128  # 16 e-chunks
DB = D * B  # 128

LAST_EXEC_TIME_NS = None
_CACHED_NC = {}

f32 = mybir.dt.float32
f8 = mybir.dt.float8e4
u32 = mybir.dt.uint32


def build_module(reps=1):
    if reps in _CACHED_NC:
        return _CACHED_NC[reps]
    import contextlib

    nc = bacc.Bacc("TRN2", target_bir_lowering=False, debug=False)

    FREE = ET * OS  # 4096 bytes per partition per delay
    dmm = nc.dram_tensor("dmm", (D, 128, ET, OS // 4), u32, kind="ExternalInput").ap()
    w8 = nc.dram_tensor("w8", (128, FREE // 4), u32, kind="ExternalInput").ap()
    sgn = nc.dram_tensor("sgn", (128, FREE // 4), u32, kind="ExternalInput").ap()
    apad = nc.dram_tensor("apad", (128, ET // 2, 2, D, 16), f8, kind="ExternalInput").ap()
    xdm = nc.dram_tensor("xdm", (128, ET * DB // 4), u32, kind="ExternalInput").ap()
    out = nc.dram_tensor("out", (B, OS), f32, kind="ExternalOutput").ap()

    with tile.TileContext(nc) as tc:
        with (
            tc.tile_pool(name="const", bufs=2) as const,
            tc.tile_pool(name="dm", bufs=6) as dmp,
            tc.tile_pool(name="m", bufs=6) as mp,
            tc.tile_pool(name="ps", bufs=2, space="PSUM") as pp,
            tc.tile_pool(name="o", bufs=2) as op,
        ):

            def body():
                w_sb = const.tile([128, FREE // 4], u32, tag="w")
                s_sb = const.tile([128, FREE // 4], u32, tag="s")
                weff = const.tile([128, ET, OS], f8, tag="weff")
                xdm_sb = const.tile([128, ET * DB // 8, 2], u32, tag="xdm")
                a8 = const.tile([128, ET // 2, 2, D, 16], f8, tag="a8")

                nc.scalar.dma_start(out=w_sb[:], in_=w8[:])
                nc.scalar.dma_start(out=s_sb[:], in_=sgn[:])
                nc.scalar.dma_start(out=a8[:], in_=apad[:])
                nc.scalar.dma_start(
                    out=xdm_sb[:], in_=xdm[:].rearrange("p (r two) -> p r two", two=2)
                )

                # weff8 = w8 | sgnmask
                nc.vector.tensor_tensor(
                    weff[:].bitcast(u32), w_sb[:], s_sb[:], mybir.AluOpType.bitwise_or
                )
                # mask lhsT cols 8:16 in place: Xd*Wshort = Wshort & Xd-mask
                a8flat = a8[:].rearrange("p tp j d c -> p (tp j d) c")
                nc.vector.tensor_tensor(
                    a8flat[:, :, B:16].bitcast(u32),
                    a8flat[:, :, B:16].bitcast(u32),
                    xdm_sb[:],
                    mybir.AluOpType.bitwise_and,
                )

                psum = pp.tile([16, OS], f32, tag="ps")
                weff32 = weff[:].bitcast(u32)
                for d in range(D):
                    dm = dmp.tile([128, ET, OS // 4], u32, tag="dm")
                    (nc.sync if d % 2 == 0 else nc.scalar).dma_start(
                        out=dm[:], in_=dmm[d]
                    )
                    m = mp.tile([128, ET, OS], f8, tag="m")
                    nc.vector.tensor_tensor(
                        m[:].bitcast(u32), dm[:], weff32, mybir.AluOpType.bitwise_and
                    )
                    for tp in range(ET // 2):
                        nc.tensor.matmul(
                            psum[:],
                            a8[:, tp, :, d, :],
                            m[:, 2 * tp : 2 * tp + 2, :],
                            start=(d == 0 and tp == 0),
                            stop=(d == D - 1 and tp == ET // 2 - 1),
                            perf_mode=mybir.MatmulPerfMode.DoubleRow,
                        )

                sb16 = op.tile([2 * B, OS], f32, tag="sb16")
                nc.scalar.copy(sb16[:], psum[:])
                hi_sb = op.tile([B, OS], f32, tag="hi")
                nc.gpsimd.dma_start(out=hi_sb[:], in_=sb16[B : 2 * B, :])
                out_sb = op.tile([B, OS], f32, tag="os")
                nc.vector.tensor_add(out_sb[:], sb16[0:B, :], hi_sb[:])
                nc.gpsimd.dma_start(out=out[:], in_=out_sb[:])

            if reps == 1:
                body()
            else:
                UNROLL = 4
                loops, rem = divmod(reps, UNROLL)
                if loops:
                    with tc.For_i(0, loops, 1, hint_engines=(mybir.EngineType.PE,)):
                        for _ in range(UNROLL):
                            body()
                for _ in range(rem):
                    body()

    nc.compile()
    _CACHED_NC[reps] = nc
    return nc


def make_in_maps(W, signs, Xd, Wshort, delaymap):
    """Host-side sharding + transport encoding (value-blind re-encodings):
    fp8/byte-mask casts and layout swizzles only."""
    import ml_dtypes

    f8n = ml_dtypes.float8_e4m3

    def swz(a2d, dtype):  # (E, X) -> [128, ET, X] with e = t*128 + p
        X = a2d.shape[1]
        return np.ascontiguousarray(
            a2d.reshape(ET, 128, X).transpose(1, 0, 2).astype(dtype)
        )

    xdT = np.transpose(Xd, (2, 0, 1)).reshape(E, DB)  # [e, d*B+b]
    wsT = np.transpose(Wshort, (2, 0, 1)).reshape(E, DB)

    def lhst_pack(a2d, dtype):
        # (E, DB) -> [128, ET/2, 2, D, B]: [p][tp][j][d][b] = a[(2tp+j)*128+p, d*B+b]
        a = a2d.reshape(ET // 2, 2, 128, D, B).transpose(2, 0, 1, 3, 4)
        return np.ascontiguousarray(a.astype(dtype))

    apad = np.concatenate(
        [lhst_pack(xdT, f8n), lhst_pack(wsT, f8n)], axis=4
    )  # [128, ET/2, 2, D, 16]: Xd | Wshort
    apad = np.ascontiguousarray(apad)
    xdm = (lhst_pack(xdT, np.uint8) * 255).reshape(128, -1).view(np.uint32)
    xdm = np.ascontiguousarray(xdm)

    in_maps = []
    for c in range(NCORES):
        sl = slice(c * OS, (c + 1) * OS)
        w8 = swz(W[:, sl], f8n).view(np.uint8).reshape(128, -1).view(np.uint32)
        sg = swz((signs[:, sl] < 0).astype(np.float32), np.uint8) * 0x80
        sg = sg.reshape(128, -1).view(np.uint32)
        dmc = np.empty((D, 128, ET, OS // 4), np.uint32)
        for d in range(D):
            mb = swz(delaymap[d][:, sl], np.uint8) * 255
            dmc[d] = mb.reshape(128, ET, OS).view(np.uint32).reshape(128, ET, OS // 4)
        in_maps.append(
            {
                "dmm": dmc,
                "w8": np.ascontiguousarray(w8),
                "sgn": np.ascontiguousarray(sg),
                "apad": apad,
                "xdm": xdm,
            }
        )
    return in_maps


def kernel(W, signs, Xd, Wshort, delaymap, trace=False):
    global LAST_EXEC_TIME_NS
    W = np.asarray(W, dtype=np.float32)
    signs = np.asarray(signs, dtype=np.float32)
    Xd = np.asarray(Xd, dtype=np.float32)
    Wshort = np.asarray(Wshort, dtype=np.float32)
    delaymap = np.asarray(delaymap, dtype=np.float32)

    nc = build_module()
    in_maps = make_in_maps(W, signs, Xd, Wshort, delaymap)
    res = run_bass_kernel_spmd(nc, in_maps, core_ids=list(range(NCORES)), trace=trace)
    LAST_EXEC_TIME_NS = res.exec_time_ns
    return np.concatenate([r["out"] for r in res.results], axis=1)
